# revision 32
# baseline (speedup 1.0000x reference)
"""Causal self-attention with bias — Trainium2 Bass kernel, 8-way sharded.

Sharding: core c -> batch b = c//2, heads h in [8*(c%2), 8*(c%2)+8).
Per core: column-split W_attn (QKV for its 8 heads), full attention for
8 (b, h) pairs, row-split W_proj partial product. Host sums the two
partials per batch and adds the (projected) biases.

All matmuls run in fp32r (4x fp32 throughput, ~1e-4 relative error).
Softmax is computed without max-subtraction (scores are O(1) for this
problem's scale) and without any partition-dim reduction: the exp'd
scores P^T live in [key, query] layout, so the denominator l[q] comes
out of the att@V matmul itself via a ones-column appended to V.

Phase 2 runs entirely in the (64,128) PE tile mode: QK matmuls for a
head PAIR run concurrently on row-tiles T0/T8 (contraction D=64), and
the att@V contraction (128 keys) is split into two 64-row halves that
accumulate into the same PSUM bank — no tile-mode switches (each switch
drains the PE), and the array's full 128 rows stay active so the HAM
clock gate keeps the PE at 2.4 GHz. Softmax normalization is deferred:
denominator rows collect into a [32, 512] tile, one batched reciprocal,
then per-(pair, j) broadcast matmuls with one-hot selector weights.
"""

import math
from contextlib import ExitStack

import numpy as np

import concourse.bass as bass
import concourse.mybir as mybir
from concourse import bacc
from concourse.bass_utils import run_bass_kernel_spmd
from concourse.masks import make_identity
from concourse.tile import TileContext

B, T, C = 4, 2048, 1024
H, D = 16, 64
HL = 8            # heads per core
NCORES = 8
P = 128
CK = C // P       # 8 contraction chunks for the QKV projection
TB = 512          # t-block (query-block) width
NTB = T // TB     # 4
NTT = T // P      # 16 row tiles
QKC = 2 * HL * D  # 1024 q+k channels per core
VC = HL * D       # 512 v channels per core
PC = VC           # 512 proj contraction channels per core
NPAIR = HL // 2   # 4 head pairs per core

f32 = mybir.dt.float32
f32r = mybir.dt.float32r


def _build_program():
    nc = bacc.Bacc("TRN2", target_bir_lowering=False, debug=False)
    # fp32r is bit-compatible with fp32 (HW rounds on read) — declaring the
    # inputs as fp32r lets DMA feed matmul tiles directly, no cast pass.
    x = nc.dram_tensor("x", (T, C), f32r, kind="ExternalInput").ap()
    wqkv = nc.dram_tensor("wqkv", (C, 3 * VC), f32r, kind="ExternalInput").ap()
    bqk = nc.dram_tensor("bqk", (P, CK), f32, kind="ExternalInput").ap()
    wproj = nc.dram_tensor("wproj", (PC, C), f32r, kind="ExternalInput").ap()
    # one-hot selector weights for the denominator broadcast matmuls:
    # sel[k, v, c] = 1 iff k == 2v + (c >= 64)
    sel = nc.dram_tensor("sel", (32, 2 * HL * P), f32r, kind="ExternalInput").ap()
    out = nc.dram_tensor("out", (T, C), f32, kind="ExternalOutput").ap()

    with TileContext(nc) as tc:
        with ExitStack() as ctx:
            # ---- persistent pools (whole kernel) ----
            const = ctx.enter_context(tc.tile_pool(name="const", bufs=1))
            persist = ctx.enter_context(tc.tile_pool(name="persist", bufs=1))

            identity = const.tile([P, P], f32)
            make_identity(nc, identity)
            identity_r = const.tile([P, P], f32r)
            nc.vector.tensor_copy(identity_r[:], identity[:])
            # causal mask bank: mw[p, i] = 1.0 iff i >= p + 512. The slice
            # mw[:, 512-dlt : 640] multiplied into exp'd scores zeroes
            # key > query entries for a chunk whose diagonal sits at dlt.
            mw = const.tile([P, 5 * P], f32)
            nc.gpsimd.memset(mw[:], 1.0)
            nc.gpsimd.affine_select(
                out=mw[:],
                in_=mw[:],
                compare_op=mybir.AluOpType.is_ge,
                fill=0.0,
                base=-512,
                pattern=[[1, 5 * P]],
                channel_multiplier=-1,
            )
            ones_f = const.tile([P, 1], f32)
            nc.gpsimd.memset(ones_f[:], 1.0)
            bqk_sb = const.tile([P, CK], f32)
            nc.sync.dma_start(bqk_sb[:], bqk)
            sel_sb = const.tile([32, 2 * HL, P], f32r)
            nc.sync.dma_start(sel_sb[:], sel.rearrange("k (v c) -> k v c", c=P))

            # V (no bias, fp32r) resident for phase 2: [t-part, tt, head, 64+1]
            vaug = persist.tile([P, NTT, HL, D + 1], f32r)
            nc.vector.tensor_copy(
                vaug[:, :, :, D : D + 1],
                ones_f[:, None, None, 0:1].to_broadcast((P, NTT, HL, 1)),
            )
            # softmax denominators, row 2*(4m+j) = head 2m, +1 = head 2m+1
            lrows = persist.tile([2 * NPAIR * NTB, TB], f32)

            with ExitStack() as c12:
                # Q^T/K^T resident across phases 1-2:
                # [128 rows = 2 heads x 64 d, jtile 0..3 = Q pairs,
                #  jtile 4..7 = K pairs, t]
                qkt_pool = c12.enter_context(tc.tile_pool(name="qkt", bufs=1))
                qkt = qkt_pool.tile([P, CK, T], f32r)

                # ---- phase 1: x^T, QKV^T ----
                with ExitStack() as c1:
                    wq_pool = c1.enter_context(tc.tile_pool(name="wq", bufs=1))
                    xin_pool = c1.enter_context(tc.tile_pool(name="xin", bufs=2))
                    xtr_pool = c1.enter_context(tc.tile_pool(name="xtr", bufs=2))
                    tp_psum = c1.enter_context(
                        tc.tile_pool(name="tp_psum", bufs=2, space="PSUM")
                    )
                    mm_psum = c1.enter_context(
                        tc.tile_pool(name="mm_psum", bufs=3, space="PSUM")
                    )

                    wqkv_r = wq_pool.tile([P, CK, 3 * VC], f32r)
                    for cc in range(CK):
                        nc.scalar.dma_start(
                            wqkv_r[:, cc, :], wqkv[cc * P : (cc + 1) * P, :]
                        )

                    def transpose_block(tb, xtr):
                        """Issue x loads for t-block tb; return closures that
                        each emit one PE transpose + ACT psum evacuation.
                        The closures are interleaved into the PREVIOUS
                        t-block's GEMM stream so the PE's MAC activity never
                        dips long enough for the HAM clock gate to throttle
                        the array back to 1.2 GHz."""
                        ops = []
                        for half in range(2):
                            t0 = tb * TB + half * 2 * P
                            xin = xin_pool.tile([P, 2, C], f32r)
                            nc.sync.dma_start(
                                xin[:],
                                x[t0 : t0 + 2 * P, :].rearrange(
                                    "(a p) c -> p a c", p=P
                                ),
                            )
                            for a in range(2):
                                for cc in range(CK):
                                    def op(xin=xin, a=a, cc=cc, half=half):
                                        tp = tp_psum.tile([P, P], f32r)
                                        nc.tensor.transpose(
                                            tp[:],
                                            xin[:, a, cc * P : (cc + 1) * P],
                                            identity_r[:],
                                        )
                                        nc.scalar.copy(
                                            xtr[
                                                :, cc,
                                                (half * 2 + a) * P
                                                : (half * 2 + a + 1) * P,
                                            ],
                                            tp[:],
                                        )
                                    ops.append(op)
                        return ops

                    def gemm_groups(tb, xtr):
                        groups = []
                        for j in range(QKC // P):
                            def gq(j=j, xtr=xtr):
                                ps = mm_psum.tile([P, TB], f32)
                                for cc in range(CK):
                                    nc.tensor.matmul(
                                        ps[:],
                                        wqkv_r[:, cc, j * P : (j + 1) * P],
                                        xtr[:, cc, :],
                                        start=(cc == 0),
                                        stop=(cc == CK - 1),
                                    )
                                nc.vector.tensor_scalar_add(
                                    qkt[:, j, tb * TB : (tb + 1) * TB],
                                    ps[:],
                                    bqk_sb[:, j : j + 1],
                                )
                            groups.append(gq)
                        for ts4 in range(TB // P):
                            def gv(ts4=ts4, tb=tb, xtr=xtr):
                                tt = tb * (TB // P) + ts4
                                ps = mm_psum.tile([P, VC], f32)
                                for cc in range(CK):
                                    nc.tensor.matmul(
                                        ps[:],
                                        xtr[:, cc, ts4 * P : (ts4 + 1) * P],
                                        wqkv_r[:, cc, QKC : QKC + VC],
                                        start=(cc == 0),
                                        stop=(cc == CK - 1),
                                    )
                                nc.vector.tensor_copy(
                                    vaug[:, tt, :, 0:D],
                                    ps[:].rearrange("p (h d) -> p h d", h=HL),
                                )
                            groups.append(gv)
                        return groups

                    xtr_cur = xtr_pool.tile([P, CK, TB], f32r, tag="xtr")
                    for op in transpose_block(0, xtr_cur):
                        op()
                    for tb in range(NTB):
                        if tb + 1 < NTB:
                            xtr_next = xtr_pool.tile([P, CK, TB], f32r, tag="xtr")
                            pend = transpose_block(tb + 1, xtr_next)
                        else:
                            xtr_next, pend = None, []
                        groups = gemm_groups(tb, xtr_cur)
                        per = -(-len(pend) // len(groups)) if pend else 0
                        for gi, g in enumerate(groups):
                            g()
                            for op in pend[gi * per : (gi + 1) * per]:
                                op()
                        xtr_cur = xtr_next

                # ---- phase 2: attention per head pair, (64,128) tile mode ----
                with ExitStack() as c2:
                    yt_pool = c2.enter_context(tc.tile_pool(name="yt", bufs=1))
                    # y^T (unnormalized): rows = head channel (2 heads per 128)
                    ytile = yt_pool.tile([P, NPAIR, T], f32r)
                    # prefetch W_proj during attention (ACT dma queue)
                    wp_pool = c2.enter_context(tc.tile_pool(name="wp", bufs=1))
                    wproj_r = wp_pool.tile([P, PC // P, C], f32r)
                    nc.scalar.dma_start(
                        wproj_r[:], wproj.rearrange("(a p) o -> p a o", p=P)
                    )

                    c2p = c2.enter_context(ExitStack())
                    pt_pool = c2p.enter_context(tc.tile_pool(name="pt", bufs=8))
                    st_pool = c2p.enter_context(tc.tile_pool(name="st", bufs=2))
                    ps_psum = c2p.enter_context(
                        tc.tile_pool(name="ps_psum", bufs=2, space="PSUM")
                    )
                    py_psum = c2p.enter_context(
                        tc.tile_pool(name="py_psum", bufs=1, space="PSUM")
                    )

                    scale = 1.0 / math.sqrt(D)
                    for m in range(NPAIR):
                        # head A = 2m on rows 0-63 / tile T0,
                        # head B = 2m+1 on rows 64-127 / tile T8
                        qT = qkt[:, m, :]
                        kT = qkt[:, 4 + m, :]
                        for j in range(NTB):
                            nch = 4 * j + 4  # causal: key chunks 0..4j+3
                            pyA0 = py_psum.tile([D + 1, TB], f32, tag="pyA0")
                            pyA1 = py_psum.tile([D + 1, TB], f32, tag="pyA1")
                            pyB0 = py_psum.tile([D + 1, TB], f32, tag="pyB0")
                            pyB1 = py_psum.tile([D + 1, TB], f32, tag="pyB1")
                            pts = []

                            def dstart(c, j=j):
                                return max(0, (c - 4 * j) * P)

                            def av_pair(c, last):
                                # att@V, contraction split across row-tiles
                                # T0 (keys 0-63) / T8 (keys 64-127) into
                                # separate PSUM tiles (cross-tile PSUM
                                # accumulation crashes); A-T0 || B-T8 then
                                # A-T8 || B-T0 keeps both tiles streaming.
                                d0 = dstart(c)
                                pt = pts[c]
                                first = c == 0
                                nc.tensor.matmul(
                                    pyA0[:, d0:],
                                    vaug[0:64, c, 2 * m, :],
                                    pt[0:64, 0, d0:],
                                    start=first, stop=last,
                                )
                                nc.tensor.matmul(
                                    pyB1[:, d0:],
                                    vaug[64:128, c, 2 * m + 1, :],
                                    pt[64:128, 1, d0:],
                                    start=first, stop=last,
                                )
                                nc.tensor.matmul(
                                    pyA1[:, d0:],
                                    vaug[64:128, c, 2 * m, :],
                                    pt[64:128, 0, d0:],
                                    start=first, stop=last,
                                )
                                nc.tensor.matmul(
                                    pyB0[:, d0:],
                                    vaug[0:64, c, 2 * m + 1, :],
                                    pt[0:64, 1, d0:],
                                    start=first, stop=last,
                                )

                            for c in range(nch):
                                d0 = dstart(c)
                                ps = ps_psum.tile([P, 2, TB], f32)
                                # QK for both heads: concurrent row-tiles
                                nc.tensor.matmul(
                                    ps[:, 0, d0:],
                                    kT[0:64, c * P : (c + 1) * P],
                                    qT[0:64, j * TB + d0 : (j + 1) * TB],
                                    start=True, stop=True,
                                )
                                nc.tensor.matmul(
                                    ps[:, 1, d0:],
                                    kT[64:128, c * P : (c + 1) * P],
                                    qT[64:128, j * TB + d0 : (j + 1) * TB],
                                    start=True, stop=True,
                                )
                                pt = pt_pool.tile([P, 2, TB], f32r)
                                nc.scalar.activation(
                                    pt[:, :, d0:], ps[:, :, d0:],
                                    mybir.ActivationFunctionType.Exp, scale=scale,
                                )
                                if (c - 4 * j) * P >= 0:
                                    # zero key > query entries on the diagonal
                                    # (gpsimd: keeps the DVE off this chain)
                                    for s_ in range(2):
                                        nc.gpsimd.affine_select(
                                            out=pt[:, s_, d0 : d0 + P],
                                            in_=pt[:, s_, d0 : d0 + P],
                                            compare_op=mybir.AluOpType.is_ge,
                                            fill=0.0,
                                            base=0,
                                            pattern=[[1, P]],
                                            channel_multiplier=-1,
                                        )
                                pts.append(pt)
                                # trail AV by 4 chunks: queues enough QK/exp
                                # ahead of the py-gated first AV that the
                                # in-order PE queue doesn't head-block (and
                                # starve ACT) while the previous block's
                                # evacuation frees the py tiles.
                                if c >= 6:
                                    av_pair(c - 6, last=False)
                            for c in range(max(0, nch - 6), nch):
                                av_pair(c, last=(c == nch - 1))

                            # stash denominator rows: engine-copy to a
                            # partition-0 staging tile (engines can't write at
                            # arbitrary partition offsets), then DMA-scatter
                            # into lrows partitions r, r+1
                            r = 2 * (4 * m + j)
                            st = st_pool.tile([1, 2, TB], f32)
                            nc.vector.tensor_copy(
                                st[0:1, 0, :], pyA0[D : D + 1, :]
                            )
                            nc.vector.tensor_add(
                                st[0:1, 0, :], st[0:1, 0, :],
                                pyA1[D : D + 1, :],
                            )
                            nc.vector.tensor_copy(
                                st[0:1, 1, :], pyB0[D : D + 1, :]
                            )
                            nc.vector.tensor_add(
                                st[0:1, 1, :], st[0:1, 1, :],
                                pyB1[D : D + 1, :],
                            )
                            nc.sync.dma_start(
                                lrows[r : r + 2, :],
                                st[0:1, :, :],
                            )
                            ysA = ytile[0:64, m, j * TB : (j + 1) * TB]
                            nc.scalar.copy(ysA, pyA0[0:D, :])
                            nc.vector.tensor_add(ysA, ysA, pyA1[0:D, :])
                            ysB = ytile[64:128, m, j * TB : (j + 1) * TB]
                            nc.vector.tensor_copy(ysB, pyB0[0:D, :])
                            nc.vector.tensor_add(ysB, ysB, pyB1[0:D, :])

                    c2p.close()

                    # ---- normalize: one batched reciprocal, then per-(m,j)
                    # broadcast matmuls with one-hot selector weights ----
                    with ExitStack() as cn:
                        sm_pool = cn.enter_context(tc.tile_pool(name="sm", bufs=1))
                        lb_psum = cn.enter_context(
                            tc.tile_pool(name="lb_psum", bufs=4, space="PSUM")
                        )
                        linv_r = sm_pool.tile([2 * NPAIR * NTB, TB], f32r)
                        # f32r output is bit-identical to f32 — no precision loss
                        with nc.allow_low_precision(reason="f32r == f32 bits"):
                            nc.vector.reciprocal(linv_r[:], lrows[:])
                        for m in range(NPAIR):
                            for j in range(NTB):
                                v = 4 * m + j
                                lb = lb_psum.tile([P, TB], f32)
                                # lb rows 0-63 = 1/l_A, 64-127 = 1/l_B
                                nc.tensor.matmul(
                                    lb[:], sel_sb[:, v, :], linv_r[:],
                                    start=True, stop=True,
                                )
                                nc.vector.tensor_mul(
                                    ytile[:, m, j * TB : (j + 1) * TB],
                                    ytile[:, m, j * TB : (j + 1) * TB],
                                    lb[:],
                                )

                    # ---- phase 3: projection (row-split partial product) ----
                    with ExitStack() as c3:
                        ot_pool = c3.enter_context(tc.tile_pool(name="ot", bufs=3))
                        po_psum = c3.enter_context(
                            tc.tile_pool(name="po_psum", bufs=4, space="PSUM")
                        )
                        for t2 in range(NTT // 2):
                            ot = ot_pool.tile([P, 2, C], f32)
                            for a2 in range(2):
                                tt = 2 * t2 + a2
                                for nh in range(C // TB):
                                    po = po_psum.tile([P, TB], f32)
                                    for a in range(PC // P):
                                        nc.tensor.matmul(
                                            po[:],
                                            ytile[:, a, tt * P : (tt + 1) * P],
                                            wproj_r[
                                                :, a, nh * TB : (nh + 1) * TB
                                            ],
                                            start=(a == 0),
                                            stop=(a == PC // P - 1),
                                        )
                                    nc.vector.tensor_copy(
                                        ot[:, a2, nh * TB : (nh + 1) * TB], po[:]
                                    )
                            (nc.sync if t2 % 2 == 0 else nc.scalar).dma_start(
                                out[2 * t2 * P : (2 * t2 + 2) * P, :].rearrange(
                                    "(a p) c -> p a c", p=P
                                ),
                                ot[:],
                            )

    nc.compile()
    return nc


_NC_CACHE = None


def _get_program():
    global _NC_CACHE
    if _NC_CACHE is None:
        _NC_CACHE = _build_program()
    return _NC_CACHE


def _sel_matrix():
    # sel[k, v, c] = 1 iff k == 2v + (c >= 64); broadcast-matmul weights
    # that map denominator rows [32, TB] to a per-(pair, j) [128, TB] tile.
    s = np.zeros((32, 2 * HL, P), dtype=np.float32)
    for v in range(2 * HL):
        s[2 * v, v, 0:64] = 1.0
        s[2 * v + 1, v, 64:128] = 1.0
    return np.ascontiguousarray(s.reshape(32, 2 * HL * P))


def _shard_inputs(x, W_attn, b_attn, bQ, bK, bV, W_proj):
    sel = _sel_matrix()
    in_maps = []
    for c in range(NCORES):
        b = c // 2
        half = c % 2
        s = half * VC
        wq = W_attn[:, s : s + VC]
        wk = W_attn[:, C + s : C + s + VC]
        wv = W_attn[:, 2 * C + s : 2 * C + s + VC]
        wqkv = np.ascontiguousarray(np.concatenate([wq, wk, wv], axis=1))
        bq = b_attn[s : s + VC] + bQ[half * HL : half * HL + HL].reshape(-1)
        bk = b_attn[C + s : C + s + VC] + bK[half * HL : half * HL + HL].reshape(-1)
        bqk = np.ascontiguousarray(
            np.concatenate([bq, bk]).reshape(CK, P).T.astype(np.float32)
        )
        wproj = np.ascontiguousarray(W_proj[s : s + VC, :])
        in_maps.append(
            {
                "x": np.ascontiguousarray(x[b]),
                "wqkv": wqkv,
                "bqk": bqk,
                "wproj": wproj,
                "sel": sel,
            }
        )
    return in_maps


def kernel(x, W_attn, b_attn, W_proj, b_proj, bQ, bK, bV, _trace=False, _res_out=None):
    x = np.asarray(x, dtype=np.float32)
    W_attn = np.asarray(W_attn, dtype=np.float32)
    b_attn = np.asarray(b_attn, dtype=np.float32)
    W_proj = np.asarray(W_proj, dtype=np.float32)
    b_proj = np.asarray(b_proj, dtype=np.float32)
    bQ = np.asarray(bQ, dtype=np.float32)
    bK = np.asarray(bK, dtype=np.float32)
    bV = np.asarray(bV, dtype=np.float32)

    nc = _get_program()
    in_maps = _shard_inputs(x, W_attn, b_attn, bQ, bK, bV, W_proj)
    res = run_bass_kernel_spmd(
        nc, in_maps, core_ids=list(range(NCORES)), trace=_trace
    )
    if _res_out is not None:
        _res_out.append(res)

    # v-bias passes through softmax untouched (rows of att sum to 1), so it
    # projects to a constant vector; fold it with b_proj on the host.
    bv = b_attn[2 * C : 3 * C] + bV.reshape(-1)
    extra = bv @ W_proj + b_proj
    out = np.empty((B, T, C), dtype=np.float32)
    for b in range(B):
        out[b] = res.results[2 * b]["out"] + res.results[2 * b + 1]["out"] + extra
    return out


# revision 35
# speedup vs baseline: 1.0122x; 1.0122x over previous
"""Causal self-attention with bias — Trainium2 Bass kernel, 8-way sharded.

Sharding: core c -> batch b = c//2, heads h in [8*(c%2), 8*(c%2)+8).
Per core: column-split W_attn (QKV for its 8 heads), full attention for
8 (b, h) pairs, row-split W_proj partial product. Host sums the two
partials per batch and adds the (projected) biases.

All matmuls run in fp32r (4x fp32 throughput, ~1e-4 relative error).
Softmax is computed without max-subtraction (scores are O(1) for this
problem's scale) and without any partition-dim reduction: the exp'd
scores P^T live in [key, query] layout, so the denominator l[q] comes
out of the att@V matmul itself via a ones-column appended to V.

Phase 2 runs entirely in the (64,128) PE tile mode: QK matmuls for a
head PAIR run concurrently on row-tiles T0/T8 (contraction D=64), and
the att@V contraction (128 keys) is split into two 64-row halves that
accumulate into the same PSUM bank — no tile-mode switches (each switch
drains the PE), and the array's full 128 rows stay active so the HAM
clock gate keeps the PE at 2.4 GHz. Softmax normalization is deferred:
denominator rows collect into a [32, 512] tile, one batched reciprocal,
then per-(pair, j) broadcast matmuls with one-hot selector weights.
"""

import math
from contextlib import ExitStack

import numpy as np

import concourse.bass as bass
import concourse.mybir as mybir
from concourse import bacc
from concourse.bass_utils import run_bass_kernel_spmd
from concourse.masks import make_identity
from concourse.tile import TileContext

B, T, C = 4, 2048, 1024
H, D = 16, 64
HL = 8            # heads per core
NCORES = 8
P = 128
CK = C // P       # 8 contraction chunks for the QKV projection
TB = 512          # t-block (query-block) width
NTB = T // TB     # 4
NTT = T // P      # 16 row tiles
QKC = 2 * HL * D  # 1024 q+k channels per core
VC = HL * D       # 512 v channels per core
PC = VC           # 512 proj contraction channels per core
NPAIR = HL // 2   # 4 head pairs per core

f32 = mybir.dt.float32
f32r = mybir.dt.float32r


def _build_program():
    nc = bacc.Bacc("TRN2", target_bir_lowering=False, debug=False)
    # fp32r is bit-compatible with fp32 (HW rounds on read) — declaring the
    # inputs as fp32r lets DMA feed matmul tiles directly, no cast pass.
    x = nc.dram_tensor("x", (T, C), f32r, kind="ExternalInput").ap()
    wqkv = nc.dram_tensor("wqkv", (C, 3 * VC), f32r, kind="ExternalInput").ap()
    bqk = nc.dram_tensor("bqk", (P, CK), f32, kind="ExternalInput").ap()
    wproj = nc.dram_tensor("wproj", (PC, C), f32r, kind="ExternalInput").ap()
    # one-hot selector weights for the denominator broadcast matmuls:
    # sel[k, v, c] = 1 iff k == 2v + (c >= 64)
    sel = nc.dram_tensor("sel", (32, 2 * HL * P), f32r, kind="ExternalInput").ap()
    out = nc.dram_tensor("out", (T, C), f32, kind="ExternalOutput").ap()

    with TileContext(nc) as tc:
        with ExitStack() as ctx:
            # ---- persistent pools (whole kernel) ----
            const = ctx.enter_context(tc.tile_pool(name="const", bufs=1))
            persist = ctx.enter_context(tc.tile_pool(name="persist", bufs=1))

            identity = const.tile([P, P], f32)
            make_identity(nc, identity)
            identity_r = const.tile([P, P], f32r)
            nc.vector.tensor_copy(identity_r[:], identity[:])
            # causal mask bank: mw[p, i] = 1.0 iff i >= p + 512. The slice
            # mw[:, 512-dlt : 640] multiplied into exp'd scores zeroes
            # key > query entries for a chunk whose diagonal sits at dlt.
            mw = const.tile([P, 5 * P], f32)
            nc.gpsimd.memset(mw[:], 1.0)
            nc.gpsimd.affine_select(
                out=mw[:],
                in_=mw[:],
                compare_op=mybir.AluOpType.is_ge,
                fill=0.0,
                base=-512,
                pattern=[[1, 5 * P]],
                channel_multiplier=-1,
            )
            ones_f = const.tile([P, 1], f32)
            nc.gpsimd.memset(ones_f[:], 1.0)
            bqk_sb = const.tile([P, CK], f32)
            nc.sync.dma_start(bqk_sb[:], bqk)
            sel_sb = const.tile([32, 2 * HL, P], f32r)
            nc.sync.dma_start(sel_sb[:], sel.rearrange("k (v c) -> k v c", c=P))

            # V (no bias, fp32r) resident for phase 2: [t-part, tt, head, 64+1]
            vaug = persist.tile([P, NTT, HL, D + 1], f32r)
            nc.vector.tensor_copy(
                vaug[:, :, :, D : D + 1],
                ones_f[:, None, None, 0:1].to_broadcast((P, NTT, HL, 1)),
            )
            # softmax denominators, row 2*(4m+j) = head 2m, +1 = head 2m+1
            lrows = persist.tile([2 * NPAIR * NTB, TB], f32)

            with ExitStack() as c12:
                # Q^T/K^T resident across phases 1-2:
                # [128 rows = 2 heads x 64 d, jtile 0..3 = Q pairs,
                #  jtile 4..7 = K pairs, t]
                qkt_pool = c12.enter_context(tc.tile_pool(name="qkt", bufs=1))
                qkt = qkt_pool.tile([P, CK, T], f32r)

                # ---- phase 1: x^T, QKV^T ----
                with ExitStack() as c1:
                    wq_pool = c1.enter_context(tc.tile_pool(name="wq", bufs=1))
                    xin_pool = c1.enter_context(tc.tile_pool(name="xin", bufs=2))
                    xtr_pool = c1.enter_context(tc.tile_pool(name="xtr", bufs=2))
                    tp_psum = c1.enter_context(
                        tc.tile_pool(name="tp_psum", bufs=3, space="PSUM")
                    )
                    mm_psum = c1.enter_context(
                        tc.tile_pool(name="mm_psum", bufs=5, space="PSUM")
                    )

                    wqkv_r = wq_pool.tile([P, CK, 3 * VC], f32r)
                    for cc in range(CK):
                        nc.scalar.dma_start(
                            wqkv_r[:, cc, :], wqkv[cc * P : (cc + 1) * P, :]
                        )

                    def transpose_block(tb, xtr):
                        """Issue x loads for t-block tb; return closures that
                        each emit one PE transpose + ACT psum evacuation.
                        The closures are interleaved into the PREVIOUS
                        t-block's GEMM stream so the PE's MAC activity never
                        dips long enough for the HAM clock gate to throttle
                        the array back to 1.2 GHz."""
                        ops = []
                        for half in range(2):
                            t0 = tb * TB + half * 2 * P
                            xin = xin_pool.tile([P, 2, C], f32r)
                            nc.sync.dma_start(
                                xin[:],
                                x[t0 : t0 + 2 * P, :].rearrange(
                                    "(a p) c -> p a c", p=P
                                ),
                            )
                            for a in range(2):
                                for cc in range(CK):
                                    def op(xin=xin, a=a, cc=cc, half=half):
                                        tp = tp_psum.tile([P, P], f32r)
                                        nc.tensor.transpose(
                                            tp[:],
                                            xin[:, a, cc * P : (cc + 1) * P],
                                            identity_r[:],
                                        )
                                        nc.scalar.copy(
                                            xtr[
                                                :, cc,
                                                (half * 2 + a) * P
                                                : (half * 2 + a + 1) * P,
                                            ],
                                            tp[:],
                                        )
                                    ops.append(op)
                        return ops

                    def gemm_groups(tb, xtr):
                        groups = []
                        for j in range(QKC // P):
                            def gq(j=j, xtr=xtr):
                                ps = mm_psum.tile([P, TB], f32, tag="mm")
                                for cc in range(CK):
                                    nc.tensor.matmul(
                                        ps[:],
                                        wqkv_r[:, cc, j * P : (j + 1) * P],
                                        xtr[:, cc, :],
                                        start=(cc == 0),
                                        stop=(cc == CK - 1),
                                    )
                                nc.vector.tensor_scalar_add(
                                    qkt[:, j, tb * TB : (tb + 1) * TB],
                                    ps[:],
                                    bqk_sb[:, j : j + 1],
                                )
                            groups.append(gq)
                        for ts4 in range(TB // P):
                            def gv(ts4=ts4, tb=tb, xtr=xtr):
                                tt = tb * (TB // P) + ts4
                                ps = mm_psum.tile([P, VC], f32, tag="mm")
                                for cc in range(CK):
                                    nc.tensor.matmul(
                                        ps[:],
                                        xtr[:, cc, ts4 * P : (ts4 + 1) * P],
                                        wqkv_r[:, cc, QKC : QKC + VC],
                                        start=(cc == 0),
                                        stop=(cc == CK - 1),
                                    )
                                nc.vector.tensor_copy(
                                    vaug[:, tt, :, 0:D],
                                    ps[:].rearrange("p (h d) -> p h d", h=HL),
                                )
                            groups.append(gv)
                        return groups

                    xtr_cur = xtr_pool.tile([P, CK, TB], f32r, tag="xtr")
                    for op in transpose_block(0, xtr_cur):
                        op()
                    for tb in range(NTB):
                        if tb + 1 < NTB:
                            xtr_next = xtr_pool.tile([P, CK, TB], f32r, tag="xtr")
                            pend = transpose_block(tb + 1, xtr_next)
                        else:
                            xtr_next, pend = None, []
                        groups = gemm_groups(tb, xtr_cur)
                        per = -(-len(pend) // len(groups)) if pend else 0
                        for gi, g in enumerate(groups):
                            g()
                            for op in pend[gi * per : (gi + 1) * per]:
                                op()
                        xtr_cur = xtr_next

                # ---- phase 2: attention per head pair, (64,128) tile mode ----
                with ExitStack() as c2:
                    yt_pool = c2.enter_context(tc.tile_pool(name="yt", bufs=1))
                    # y^T (unnormalized): rows = head channel (2 heads per 128)
                    ytile = yt_pool.tile([P, NPAIR, T], f32r)
                    # prefetch W_proj during attention (ACT dma queue)
                    wp_pool = c2.enter_context(tc.tile_pool(name="wp", bufs=1))
                    wproj_r = wp_pool.tile([P, PC // P, C], f32r)
                    nc.scalar.dma_start(
                        wproj_r[:], wproj.rearrange("(a p) o -> p a o", p=P)
                    )

                    c2p = c2.enter_context(ExitStack())
                    pt_pool = c2p.enter_context(tc.tile_pool(name="pt", bufs=6))
                    st_pool = c2p.enter_context(tc.tile_pool(name="st", bufs=2))
                    ps_psum = c2p.enter_context(
                        tc.tile_pool(name="ps_psum", bufs=2, space="PSUM")
                    )
                    py_psum = c2p.enter_context(
                        tc.tile_pool(name="py_psum", bufs=1, space="PSUM")
                    )

                    scale = 1.0 / math.sqrt(D)
                    for m in range(NPAIR):
                        # head A = 2m on rows 0-63 / tile T0,
                        # head B = 2m+1 on rows 64-127 / tile T8
                        qT = qkt[:, m, :]
                        kT = qkt[:, 4 + m, :]
                        for j in range(NTB):
                            nch = 4 * j + 4  # causal: key chunks 0..4j+3
                            pyA0 = py_psum.tile([D + 1, TB], f32, tag="pyA0")
                            pyA1 = py_psum.tile([D + 1, TB], f32, tag="pyA1")
                            pyB0 = py_psum.tile([D + 1, TB], f32, tag="pyB0")
                            pyB1 = py_psum.tile([D + 1, TB], f32, tag="pyB1")
                            pts = []

                            def dstart(c, j=j):
                                return max(0, (c - 4 * j) * P)

                            def av_pair(c, last):
                                # att@V, contraction split across row-tiles
                                # T0 (keys 0-63) / T8 (keys 64-127) into
                                # separate PSUM tiles (cross-tile PSUM
                                # accumulation crashes); A-T0 || B-T8 then
                                # A-T8 || B-T0 keeps both tiles streaming.
                                d0 = dstart(c)
                                pt = pts[c]
                                first = c == 0
                                nc.tensor.matmul(
                                    pyA0[:, d0:],
                                    vaug[0:64, c, 2 * m, :],
                                    pt[0:64, 0, d0:],
                                    start=first, stop=last,
                                )
                                nc.tensor.matmul(
                                    pyB1[:, d0:],
                                    vaug[64:128, c, 2 * m + 1, :],
                                    pt[64:128, 1, d0:],
                                    start=first, stop=last,
                                )
                                nc.tensor.matmul(
                                    pyA1[:, d0:],
                                    vaug[64:128, c, 2 * m, :],
                                    pt[64:128, 0, d0:],
                                    start=first, stop=last,
                                )
                                nc.tensor.matmul(
                                    pyB0[:, d0:],
                                    vaug[0:64, c, 2 * m + 1, :],
                                    pt[0:64, 1, d0:],
                                    start=first, stop=last,
                                )

                            for c in range(nch):
                                d0 = dstart(c)
                                ps = ps_psum.tile([P, 2, TB], f32)
                                # QK for both heads: concurrent row-tiles
                                nc.tensor.matmul(
                                    ps[:, 0, d0:],
                                    kT[0:64, c * P : (c + 1) * P],
                                    qT[0:64, j * TB + d0 : (j + 1) * TB],
                                    start=True, stop=True,
                                )
                                nc.tensor.matmul(
                                    ps[:, 1, d0:],
                                    kT[64:128, c * P : (c + 1) * P],
                                    qT[64:128, j * TB + d0 : (j + 1) * TB],
                                    start=True, stop=True,
                                )
                                pt = pt_pool.tile([P, 2, TB], f32r)
                                nc.scalar.activation(
                                    pt[:, :, d0:], ps[:, :, d0:],
                                    mybir.ActivationFunctionType.Exp, scale=scale,
                                )
                                if (c - 4 * j) * P >= 0:
                                    # zero key > query entries on the diagonal
                                    # (gpsimd: keeps the DVE off this chain)
                                    for s_ in range(2):
                                        nc.gpsimd.affine_select(
                                            out=pt[:, s_, d0 : d0 + P],
                                            in_=pt[:, s_, d0 : d0 + P],
                                            compare_op=mybir.AluOpType.is_ge,
                                            fill=0.0,
                                            base=0,
                                            pattern=[[1, P]],
                                            channel_multiplier=-1,
                                        )
                                pts.append(pt)
                                # trail AV by 4 chunks: queues enough QK/exp
                                # ahead of the py-gated first AV that the
                                # in-order PE queue doesn't head-block (and
                                # starve ACT) while the previous block's
                                # evacuation frees the py tiles.
                                if c >= 4:
                                    av_pair(c - 4, last=False)
                            for c in range(max(0, nch - 4), nch):
                                av_pair(c, last=(c == nch - 1))

                            # stash denominator rows: engine-copy to a
                            # partition-0 staging tile (engines can't write at
                            # arbitrary partition offsets), then DMA-scatter
                            # into lrows partitions r, r+1
                            r = 2 * (4 * m + j)
                            st = st_pool.tile([1, 2, TB], f32)
                            nc.vector.tensor_copy(
                                st[0:1, 0, :], pyA0[D : D + 1, :]
                            )
                            nc.vector.tensor_add(
                                st[0:1, 0, :], st[0:1, 0, :],
                                pyA1[D : D + 1, :],
                            )
                            nc.vector.tensor_copy(
                                st[0:1, 1, :], pyB0[D : D + 1, :]
                            )
                            nc.vector.tensor_add(
                                st[0:1, 1, :], st[0:1, 1, :],
                                pyB1[D : D + 1, :],
                            )
                            nc.sync.dma_start(
                                lrows[r : r + 2, :],
                                st[0:1, :, :],
                            )
                            ysA = ytile[0:64, m, j * TB : (j + 1) * TB]
                            nc.scalar.copy(ysA, pyA0[0:D, :])
                            nc.vector.tensor_add(ysA, ysA, pyA1[0:D, :])
                            ysB = ytile[64:128, m, j * TB : (j + 1) * TB]
                            nc.vector.tensor_copy(ysB, pyB0[0:D, :])
                            nc.vector.tensor_add(ysB, ysB, pyB1[0:D, :])

                    c2p.close()

                    # ---- normalize: one batched reciprocal, then per-(m,j)
                    # broadcast matmuls with one-hot selector weights ----
                    with ExitStack() as cn:
                        sm_pool = cn.enter_context(tc.tile_pool(name="sm", bufs=1))
                        lb_psum = cn.enter_context(
                            tc.tile_pool(name="lb_psum", bufs=4, space="PSUM")
                        )
                        linv_r = sm_pool.tile([2 * NPAIR * NTB, TB], f32r)
                        # f32r output is bit-identical to f32 — no precision loss
                        with nc.allow_low_precision(reason="f32r == f32 bits"):
                            nc.vector.reciprocal(linv_r[:], lrows[:])
                        for m in range(NPAIR):
                            for j in range(NTB):
                                v = 4 * m + j
                                lb = lb_psum.tile([P, TB], f32)
                                # lb rows 0-63 = 1/l_A, 64-127 = 1/l_B
                                nc.tensor.matmul(
                                    lb[:], sel_sb[:, v, :], linv_r[:],
                                    start=True, stop=True,
                                )
                                nc.vector.tensor_mul(
                                    ytile[:, m, j * TB : (j + 1) * TB],
                                    ytile[:, m, j * TB : (j + 1) * TB],
                                    lb[:],
                                )

                    # ---- phase 3: projection (row-split partial product) ----
                    with ExitStack() as c3:
                        ot_pool = c3.enter_context(tc.tile_pool(name="ot", bufs=3))
                        po_psum = c3.enter_context(
                            tc.tile_pool(name="po_psum", bufs=4, space="PSUM")
                        )
                        for t2 in range(NTT // 2):
                            ot = ot_pool.tile([P, 2, C], f32)
                            for a2 in range(2):
                                tt = 2 * t2 + a2
                                for nh in range(C // TB):
                                    po = po_psum.tile([P, TB], f32)
                                    for a in range(PC // P):
                                        nc.tensor.matmul(
                                            po[:],
                                            ytile[:, a, tt * P : (tt + 1) * P],
                                            wproj_r[
                                                :, a, nh * TB : (nh + 1) * TB
                                            ],
                                            start=(a == 0),
                                            stop=(a == PC // P - 1),
                                        )
                                    nc.vector.tensor_copy(
                                        ot[:, a2, nh * TB : (nh + 1) * TB], po[:]
                                    )
                            (nc.sync if t2 % 2 == 0 else nc.scalar).dma_start(
                                out[2 * t2 * P : (2 * t2 + 2) * P, :].rearrange(
                                    "(a p) c -> p a c", p=P
                                ),
                                ot[:],
                            )

    nc.compile()
    return nc


_NC_CACHE = None


def _get_program():
    global _NC_CACHE
    if _NC_CACHE is None:
        _NC_CACHE = _build_program()
    return _NC_CACHE


def _sel_matrix():
    # sel[k, v, c] = 1 iff k == 2v + (c >= 64); broadcast-matmul weights
    # that map denominator rows [32, TB] to a per-(pair, j) [128, TB] tile.
    s = np.zeros((32, 2 * HL, P), dtype=np.float32)
    for v in range(2 * HL):
        s[2 * v, v, 0:64] = 1.0
        s[2 * v + 1, v, 64:128] = 1.0
    return np.ascontiguousarray(s.reshape(32, 2 * HL * P))


def _shard_inputs(x, W_attn, b_attn, bQ, bK, bV, W_proj):
    sel = _sel_matrix()
    in_maps = []
    for c in range(NCORES):
        b = c // 2
        half = c % 2
        s = half * VC
        wq = W_attn[:, s : s + VC]
        wk = W_attn[:, C + s : C + s + VC]
        wv = W_attn[:, 2 * C + s : 2 * C + s + VC]
        wqkv = np.ascontiguousarray(np.concatenate([wq, wk, wv], axis=1))
        bq = b_attn[s : s + VC] + bQ[half * HL : half * HL + HL].reshape(-1)
        bk = b_attn[C + s : C + s + VC] + bK[half * HL : half * HL + HL].reshape(-1)
        bqk = np.ascontiguousarray(
            np.concatenate([bq, bk]).reshape(CK, P).T.astype(np.float32)
        )
        wproj = np.ascontiguousarray(W_proj[s : s + VC, :])
        in_maps.append(
            {
                "x": np.ascontiguousarray(x[b]),
                "wqkv": wqkv,
                "bqk": bqk,
                "wproj": wproj,
                "sel": sel,
            }
        )
    return in_maps


def kernel(x, W_attn, b_attn, W_proj, b_proj, bQ, bK, bV, _trace=False, _res_out=None):
    x = np.asarray(x, dtype=np.float32)
    W_attn = np.asarray(W_attn, dtype=np.float32)
    b_attn = np.asarray(b_attn, dtype=np.float32)
    W_proj = np.asarray(W_proj, dtype=np.float32)
    b_proj = np.asarray(b_proj, dtype=np.float32)
    bQ = np.asarray(bQ, dtype=np.float32)
    bK = np.asarray(bK, dtype=np.float32)
    bV = np.asarray(bV, dtype=np.float32)

    nc = _get_program()
    in_maps = _shard_inputs(x, W_attn, b_attn, bQ, bK, bV, W_proj)
    res = run_bass_kernel_spmd(
        nc, in_maps, core_ids=list(range(NCORES)), trace=_trace
    )
    if _res_out is not None:
        _res_out.append(res)

    # v-bias passes through softmax untouched (rows of att sum to 1), so it
    # projects to a constant vector; fold it with b_proj on the host.
    bv = b_attn[2 * C : 3 * C] + bV.reshape(-1)
    extra = bv @ W_proj + b_proj
    out = np.empty((B, T, C), dtype=np.float32)
    for b in range(B):
        out[b] = res.results[2 * b]["out"] + res.results[2 * b + 1]["out"] + extra
    return out


# revision 36
# speedup vs baseline: 1.0439x; 1.0313x over previous
"""Causal self-attention with bias — Trainium2 Bass kernel, 8-way sharded.

Sharding: core c -> batch b = c//2, heads h in [8*(c%2), 8*(c%2)+8).
Per core: column-split W_attn (QKV for its 8 heads), full attention for
8 (b, h) pairs, row-split W_proj partial product. Host sums the two
partials per batch and adds the (projected) biases.

All matmuls run in fp32r (4x fp32 throughput, ~1e-4 relative error).
Softmax is computed without max-subtraction (scores are O(1) for this
problem's scale) and without any partition-dim reduction: the exp'd
scores P^T live in [key, query] layout, so the denominator l[q] comes
out of the att@V matmul itself via a ones-column appended to V.

Phase 2 runs entirely in the (64,128) PE tile mode: QK matmuls for a
head PAIR run concurrently on row-tiles T0/T8 (contraction D=64), and
the att@V contraction (128 keys) is split into two 64-row halves that
accumulate into the same PSUM bank — no tile-mode switches (each switch
drains the PE), and the array's full 128 rows stay active so the HAM
clock gate keeps the PE at 2.4 GHz. Softmax normalization is deferred:
denominator rows collect into a [32, 512] tile, one batched reciprocal,
then per-(pair, j) broadcast matmuls with one-hot selector weights.
"""

import math
from contextlib import ExitStack

import numpy as np

import concourse.bass as bass
import concourse.mybir as mybir
from concourse import bacc
from concourse.bass_utils import run_bass_kernel_spmd
from concourse.masks import make_identity
from concourse.tile import TileContext

B, T, C = 4, 2048, 1024
H, D = 16, 64
HL = 8            # heads per core
NCORES = 8
P = 128
CK = C // P       # 8 contraction chunks for the QKV projection
TB = 512          # t-block (query-block) width
NTB = T // TB     # 4
NTT = T // P      # 16 row tiles
QKC = 2 * HL * D  # 1024 q+k channels per core
VC = HL * D       # 512 v channels per core
PC = VC           # 512 proj contraction channels per core
NPAIR = HL // 2   # 4 head pairs per core

f32 = mybir.dt.float32
f32r = mybir.dt.float32r


def _build_program():
    nc = bacc.Bacc("TRN2", target_bir_lowering=False, debug=False)
    # fp32r is bit-compatible with fp32 (HW rounds on read) — declaring the
    # inputs as fp32r lets DMA feed matmul tiles directly, no cast pass.
    x = nc.dram_tensor("x", (T, C), f32r, kind="ExternalInput").ap()
    wqkv = nc.dram_tensor("wqkv", (C, 3 * VC), f32r, kind="ExternalInput").ap()
    bqk = nc.dram_tensor("bqk", (P, CK), f32, kind="ExternalInput").ap()
    wproj = nc.dram_tensor("wproj", (PC, C), f32r, kind="ExternalInput").ap()
    # one-hot selector weights for the denominator broadcast matmuls:
    # sel[k, v, c] = 1 iff k == 2v + (c >= 64)
    sel = nc.dram_tensor("sel", (32, 2 * HL * P), f32r, kind="ExternalInput").ap()
    out = nc.dram_tensor("out", (T, C), f32, kind="ExternalOutput").ap()

    with TileContext(nc) as tc:
        with ExitStack() as ctx:
            # ---- persistent pools (whole kernel) ----
            const = ctx.enter_context(tc.tile_pool(name="const", bufs=1))
            persist = ctx.enter_context(tc.tile_pool(name="persist", bufs=1))

            identity = const.tile([P, P], f32)
            make_identity(nc, identity)
            identity_r = const.tile([P, P], f32r)
            nc.vector.tensor_copy(identity_r[:], identity[:])
            # causal mask bank: mw[p, i] = 1.0 iff i >= p + 512. The slice
            # mw[:, 512-dlt : 640] multiplied into exp'd scores zeroes
            # key > query entries for a chunk whose diagonal sits at dlt.
            mw = const.tile([P, 5 * P], f32)
            nc.gpsimd.memset(mw[:], 1.0)
            nc.gpsimd.affine_select(
                out=mw[:],
                in_=mw[:],
                compare_op=mybir.AluOpType.is_ge,
                fill=0.0,
                base=-512,
                pattern=[[1, 5 * P]],
                channel_multiplier=-1,
            )
            ones_f = const.tile([P, 1], f32)
            nc.gpsimd.memset(ones_f[:], 1.0)
            bqk_sb = const.tile([P, CK], f32)
            nc.sync.dma_start(bqk_sb[:], bqk)
            sel_sb = const.tile([32, 2 * HL, P], f32r)
            nc.sync.dma_start(sel_sb[:], sel.rearrange("k (v c) -> k v c", c=P))

            # V (no bias, fp32r) resident for phase 2: [t-part, tt, head, 64+1]
            vaug = persist.tile([P, NTT, HL, D + 1], f32r)
            nc.vector.tensor_copy(
                vaug[:, :, :, D : D + 1],
                ones_f[:, None, None, 0:1].to_broadcast((P, NTT, HL, 1)),
            )
            # softmax denominators, row 2*(4m+j) = head 2m, +1 = head 2m+1
            lrows = persist.tile([2 * NPAIR * NTB, TB], f32)

            with ExitStack() as c12:
                # Q^T/K^T resident across phases 1-2:
                # [128 rows = 2 heads x 64 d, jtile 0..3 = Q pairs,
                #  jtile 4..7 = K pairs, t]
                qkt_pool = c12.enter_context(tc.tile_pool(name="qkt", bufs=1))
                qkt = qkt_pool.tile([P, CK, T], f32r)

                # ---- phase 1: x^T, QKV^T ----
                with ExitStack() as c1:
                    wq_pool = c1.enter_context(tc.tile_pool(name="wq", bufs=1))
                    xin_pool = c1.enter_context(tc.tile_pool(name="xin", bufs=2))
                    xtr_pool = c1.enter_context(tc.tile_pool(name="xtr", bufs=2))
                    tp_psum = c1.enter_context(
                        tc.tile_pool(name="tp_psum", bufs=3, space="PSUM")
                    )
                    mm_psum = c1.enter_context(
                        tc.tile_pool(name="mm_psum", bufs=5, space="PSUM")
                    )

                    wqkv_r = wq_pool.tile([P, CK, 3 * VC], f32r)
                    for cc in range(CK):
                        nc.scalar.dma_start(
                            wqkv_r[:, cc, :], wqkv[cc * P : (cc + 1) * P, :]
                        )

                    def transpose_block(tb, xtr):
                        """Issue x loads for t-block tb; return closures that
                        each emit one PE transpose + ACT psum evacuation.
                        The closures are interleaved into the PREVIOUS
                        t-block's GEMM stream so the PE's MAC activity never
                        dips long enough for the HAM clock gate to throttle
                        the array back to 1.2 GHz."""
                        ops = []
                        for half in range(2):
                            t0 = tb * TB + half * 2 * P
                            xin = xin_pool.tile([P, 2, C], f32r)
                            nc.sync.dma_start(
                                xin[:],
                                x[t0 : t0 + 2 * P, :].rearrange(
                                    "(a p) c -> p a c", p=P
                                ),
                            )
                            for a in range(2):
                                for cc in range(CK):
                                    def op(xin=xin, a=a, cc=cc, half=half):
                                        tp = tp_psum.tile([P, P], f32r)
                                        nc.tensor.transpose(
                                            tp[:],
                                            xin[:, a, cc * P : (cc + 1) * P],
                                            identity_r[:],
                                        )
                                        nc.scalar.copy(
                                            xtr[
                                                :, cc,
                                                (half * 2 + a) * P
                                                : (half * 2 + a + 1) * P,
                                            ],
                                            tp[:],
                                        )
                                    ops.append(op)
                        return ops

                    def gemm_groups(tb, xtr):
                        groups = []
                        for j in range(QKC // P):
                            def gq(j=j, xtr=xtr):
                                ps = mm_psum.tile([P, TB], f32, tag="mm")
                                for cc in range(CK):
                                    nc.tensor.matmul(
                                        ps[:],
                                        wqkv_r[:, cc, j * P : (j + 1) * P],
                                        xtr[:, cc, :],
                                        start=(cc == 0),
                                        stop=(cc == CK - 1),
                                    )
                                nc.vector.tensor_scalar_add(
                                    qkt[:, j, tb * TB : (tb + 1) * TB],
                                    ps[:],
                                    bqk_sb[:, j : j + 1],
                                )
                            groups.append(gq)
                        for ts4 in range(TB // P):
                            def gv(ts4=ts4, tb=tb, xtr=xtr):
                                tt = tb * (TB // P) + ts4
                                ps = mm_psum.tile([P, VC], f32, tag="mm")
                                for cc in range(CK):
                                    nc.tensor.matmul(
                                        ps[:],
                                        xtr[:, cc, ts4 * P : (ts4 + 1) * P],
                                        wqkv_r[:, cc, QKC : QKC + VC],
                                        start=(cc == 0),
                                        stop=(cc == CK - 1),
                                    )
                                nc.vector.tensor_copy(
                                    vaug[:, tt, :, 0:D],
                                    ps[:].rearrange("p (h d) -> p h d", h=HL),
                                )
                            groups.append(gv)
                        return groups

                    xtr_cur = xtr_pool.tile([P, CK, TB], f32r, tag="xtr")
                    for op in transpose_block(0, xtr_cur):
                        op()
                    for tb in range(NTB):
                        if tb + 1 < NTB:
                            xtr_next = xtr_pool.tile([P, CK, TB], f32r, tag="xtr")
                            pend = transpose_block(tb + 1, xtr_next)
                        else:
                            xtr_next, pend = None, []
                        groups = gemm_groups(tb, xtr_cur)
                        per = -(-len(pend) // len(groups)) if pend else 0
                        for gi, g in enumerate(groups):
                            g()
                            for op in pend[gi * per : (gi + 1) * per]:
                                op()
                        xtr_cur = xtr_next

                # ---- phase 2: attention per head pair, (64,128) tile mode ----
                with ExitStack() as c2:
                    yt_pool = c2.enter_context(tc.tile_pool(name="yt", bufs=1))
                    # y^T (unnormalized): rows = head channel (2 heads per 128)
                    ytile = yt_pool.tile([P, NPAIR, T], f32r)
                    # prefetch W_proj during attention (ACT dma queue)
                    wp_pool = c2.enter_context(tc.tile_pool(name="wp", bufs=1))
                    wproj_r = wp_pool.tile([P, PC // P, C], f32r)
                    nc.scalar.dma_start(
                        wproj_r[:], wproj.rearrange("(a p) o -> p a o", p=P)
                    )

                    c2p = c2.enter_context(ExitStack())
                    pt_pool = c2p.enter_context(tc.tile_pool(name="pt", bufs=6))
                    st_pool = c2p.enter_context(tc.tile_pool(name="st", bufs=2))
                    scr_pool = c2p.enter_context(tc.tile_pool(name="scr", bufs=2))
                    ps_psum = c2p.enter_context(
                        tc.tile_pool(name="ps_psum", bufs=2, space="PSUM")
                    )
                    py_psum = c2p.enter_context(
                        tc.tile_pool(name="py_psum", bufs=1, space="PSUM")
                    )

                    scale = 1.0 / math.sqrt(D)
                    for m in range(NPAIR):
                        # head A = 2m on rows 0-63 / tile T0,
                        # head B = 2m+1 on rows 64-127 / tile T8
                        qT = qkt[:, m, :]
                        kT = qkt[:, 4 + m, :]
                        for j in range(NTB):
                            nch = 4 * j + 4  # causal: key chunks 0..4j+3
                            pyA0 = py_psum.tile([D + 1, TB], f32, tag="pyA0")
                            pyA1 = py_psum.tile([D + 1, TB], f32, tag="pyA1")
                            pyB0 = py_psum.tile([D + 1, TB], f32, tag="pyB0")
                            pyB1 = py_psum.tile([D + 1, TB], f32, tag="pyB1")
                            pts = []

                            def dstart(c, j=j):
                                return max(0, (c - 4 * j) * P)

                            def av_pair(c, last):
                                # att@V, contraction split across row-tiles
                                # T0 (keys 0-63) / T8 (keys 64-127) into
                                # separate PSUM tiles (cross-tile PSUM
                                # accumulation crashes); A-T0 || B-T8 then
                                # A-T8 || B-T0 keeps both tiles streaming.
                                d0 = dstart(c)
                                pt = pts[c]
                                first = c == 0
                                nc.tensor.matmul(
                                    pyA0[:, d0:],
                                    vaug[0:64, c, 2 * m, :],
                                    pt[0:64, 0, d0:],
                                    start=first, stop=last,
                                )
                                nc.tensor.matmul(
                                    pyB1[:, d0:],
                                    vaug[64:128, c, 2 * m + 1, :],
                                    pt[64:128, 1, d0:],
                                    start=first, stop=last,
                                )
                                nc.tensor.matmul(
                                    pyA1[:, d0:],
                                    vaug[64:128, c, 2 * m, :],
                                    pt[64:128, 0, d0:],
                                    start=first, stop=last,
                                )
                                nc.tensor.matmul(
                                    pyB0[:, d0:],
                                    vaug[0:64, c, 2 * m + 1, :],
                                    pt[0:64, 1, d0:],
                                    start=first, stop=last,
                                )

                            for c in range(nch):
                                d0 = dstart(c)
                                ps = ps_psum.tile([P, 2, TB], f32)
                                # QK for both heads: concurrent row-tiles
                                nc.tensor.matmul(
                                    ps[:, 0, d0:],
                                    kT[0:64, c * P : (c + 1) * P],
                                    qT[0:64, j * TB + d0 : (j + 1) * TB],
                                    start=True, stop=True,
                                )
                                nc.tensor.matmul(
                                    ps[:, 1, d0:],
                                    kT[64:128, c * P : (c + 1) * P],
                                    qT[64:128, j * TB + d0 : (j + 1) * TB],
                                    start=True, stop=True,
                                )
                                pt = pt_pool.tile([P, 2, TB], f32r)
                                nc.scalar.activation(
                                    pt[:, :, d0:], ps[:, :, d0:],
                                    mybir.ActivationFunctionType.Exp, scale=scale,
                                )
                                if (c - 4 * j) * P >= 0:
                                    # zero key > query entries on the diagonal
                                    # (gpsimd: keeps the DVE off this chain)
                                    for s_ in range(2):
                                        nc.gpsimd.affine_select(
                                            out=pt[:, s_, d0 : d0 + P],
                                            in_=pt[:, s_, d0 : d0 + P],
                                            compare_op=mybir.AluOpType.is_ge,
                                            fill=0.0,
                                            base=0,
                                            pattern=[[1, P]],
                                            channel_multiplier=-1,
                                        )
                                pts.append(pt)
                                # trail AV by 4 chunks: queues enough QK/exp
                                # ahead of the py-gated first AV that the
                                # in-order PE queue doesn't head-block (and
                                # starve ACT) while the previous block's
                                # evacuation frees the py tiles.
                                if c >= 4:
                                    av_pair(c - 4, last=False)
                            for c in range(max(0, nch - 4), nch):
                                av_pair(c, last=(c == nch - 1))

                            # stash denominator rows: engine-copy to a
                            # partition-0 staging tile (engines can't write at
                            # arbitrary partition offsets), then DMA-scatter
                            # into lrows partitions r, r+1
                            # evacuate each T0-half once (ACT takes head A
                            # incl. its denominator row, DVE head B), then
                            # fused adds with the T8-halves write ytile and
                            # the denominator staging in single passes.
                            r = 2 * (4 * m + j)
                            scrA = scr_pool.tile([D + 1, TB], f32, tag="sA")
                            scrB = scr_pool.tile([D + 1, TB], f32, tag="sB")
                            nc.scalar.copy(scrA[:], pyA0[:])
                            nc.vector.tensor_copy(scrB[:], pyB0[:])
                            st = st_pool.tile([1, 2, TB], f32)
                            nc.vector.tensor_add(
                                st[0:1, 0, :], scrA[D : D + 1, :],
                                pyA1[D : D + 1, :],
                            )
                            nc.vector.tensor_add(
                                st[0:1, 1, :], scrB[D : D + 1, :],
                                pyB1[D : D + 1, :],
                            )
                            nc.sync.dma_start(
                                lrows[r : r + 2, :],
                                st[0:1, :, :],
                            )
                            ysA = ytile[0:64, m, j * TB : (j + 1) * TB]
                            nc.vector.tensor_add(ysA, scrA[0:D, :], pyA1[0:D, :])
                            ysB = ytile[64:128, m, j * TB : (j + 1) * TB]
                            nc.vector.tensor_add(ysB, scrB[0:D, :], pyB1[0:D, :])

                    c2p.close()

                    # ---- normalize: one batched reciprocal, then per-(m,j)
                    # broadcast matmuls with one-hot selector weights ----
                    with ExitStack() as cn:
                        sm_pool = cn.enter_context(tc.tile_pool(name="sm", bufs=1))
                        lb_psum = cn.enter_context(
                            tc.tile_pool(name="lb_psum", bufs=4, space="PSUM")
                        )
                        linv_r = sm_pool.tile([2 * NPAIR * NTB, TB], f32r)
                        # f32r output is bit-identical to f32 — no precision loss
                        with nc.allow_low_precision(reason="f32r == f32 bits"):
                            nc.vector.reciprocal(linv_r[:], lrows[:])
                        for m in range(NPAIR):
                            for j in range(NTB):
                                v = 4 * m + j
                                lb = lb_psum.tile([P, TB], f32)
                                # lb rows 0-63 = 1/l_A, 64-127 = 1/l_B
                                nc.tensor.matmul(
                                    lb[:], sel_sb[:, v, :], linv_r[:],
                                    start=True, stop=True,
                                )
                                nc.vector.tensor_mul(
                                    ytile[:, m, j * TB : (j + 1) * TB],
                                    ytile[:, m, j * TB : (j + 1) * TB],
                                    lb[:],
                                )

                    # ---- phase 3: projection (row-split partial product) ----
                    with ExitStack() as c3:
                        ot_pool = c3.enter_context(tc.tile_pool(name="ot", bufs=3))
                        po_psum = c3.enter_context(
                            tc.tile_pool(name="po_psum", bufs=4, space="PSUM")
                        )
                        for t2 in range(NTT // 2):
                            ot = ot_pool.tile([P, 2, C], f32)
                            for a2 in range(2):
                                tt = 2 * t2 + a2
                                for nh in range(C // TB):
                                    po = po_psum.tile([P, TB], f32)
                                    for a in range(PC // P):
                                        nc.tensor.matmul(
                                            po[:],
                                            ytile[:, a, tt * P : (tt + 1) * P],
                                            wproj_r[
                                                :, a, nh * TB : (nh + 1) * TB
                                            ],
                                            start=(a == 0),
                                            stop=(a == PC // P - 1),
                                        )
                                    nc.vector.tensor_copy(
                                        ot[:, a2, nh * TB : (nh + 1) * TB], po[:]
                                    )
                            (nc.sync if t2 % 2 == 0 else nc.scalar).dma_start(
                                out[2 * t2 * P : (2 * t2 + 2) * P, :].rearrange(
                                    "(a p) c -> p a c", p=P
                                ),
                                ot[:],
                            )

    nc.compile()
    return nc


_NC_CACHE = None


def _get_program():
    global _NC_CACHE
    if _NC_CACHE is None:
        _NC_CACHE = _build_program()
    return _NC_CACHE


def _sel_matrix():
    # sel[k, v, c] = 1 iff k == 2v + (c >= 64); broadcast-matmul weights
    # that map denominator rows [32, TB] to a per-(pair, j) [128, TB] tile.
    s = np.zeros((32, 2 * HL, P), dtype=np.float32)
    for v in range(2 * HL):
        s[2 * v, v, 0:64] = 1.0
        s[2 * v + 1, v, 64:128] = 1.0
    return np.ascontiguousarray(s.reshape(32, 2 * HL * P))


def _shard_inputs(x, W_attn, b_attn, bQ, bK, bV, W_proj):
    sel = _sel_matrix()
    in_maps = []
    for c in range(NCORES):
        b = c // 2
        half = c % 2
        s = half * VC
        wq = W_attn[:, s : s + VC]
        wk = W_attn[:, C + s : C + s + VC]
        wv = W_attn[:, 2 * C + s : 2 * C + s + VC]
        wqkv = np.ascontiguousarray(np.concatenate([wq, wk, wv], axis=1))
        bq = b_attn[s : s + VC] + bQ[half * HL : half * HL + HL].reshape(-1)
        bk = b_attn[C + s : C + s + VC] + bK[half * HL : half * HL + HL].reshape(-1)
        bqk = np.ascontiguousarray(
            np.concatenate([bq, bk]).reshape(CK, P).T.astype(np.float32)
        )
        wproj = np.ascontiguousarray(W_proj[s : s + VC, :])
        in_maps.append(
            {
                "x": np.ascontiguousarray(x[b]),
                "wqkv": wqkv,
                "bqk": bqk,
                "wproj": wproj,
                "sel": sel,
            }
        )
    return in_maps


def kernel(x, W_attn, b_attn, W_proj, b_proj, bQ, bK, bV, _trace=False, _res_out=None):
    x = np.asarray(x, dtype=np.float32)
    W_attn = np.asarray(W_attn, dtype=np.float32)
    b_attn = np.asarray(b_attn, dtype=np.float32)
    W_proj = np.asarray(W_proj, dtype=np.float32)
    b_proj = np.asarray(b_proj, dtype=np.float32)
    bQ = np.asarray(bQ, dtype=np.float32)
    bK = np.asarray(bK, dtype=np.float32)
    bV = np.asarray(bV, dtype=np.float32)

    nc = _get_program()
    in_maps = _shard_inputs(x, W_attn, b_attn, bQ, bK, bV, W_proj)
    res = run_bass_kernel_spmd(
        nc, in_maps, core_ids=list(range(NCORES)), trace=_trace
    )
    if _res_out is not None:
        _res_out.append(res)

    # v-bias passes through softmax untouched (rows of att sum to 1), so it
    # projects to a constant vector; fold it with b_proj on the host.
    bv = b_attn[2 * C : 3 * C] + bV.reshape(-1)
    extra = bv @ W_proj + b_proj
    out = np.empty((B, T, C), dtype=np.float32)
    for b in range(B):
        out[b] = res.results[2 * b]["out"] + res.results[2 * b + 1]["out"] + extra
    return out


# revision 37
# speedup vs baseline: 1.0532x; 1.0089x over previous
"""Causal self-attention with bias — Trainium2 Bass kernel, 8-way sharded.

Sharding: core c -> batch b = c//2, heads h in [8*(c%2), 8*(c%2)+8).
Per core: column-split W_attn (QKV for its 8 heads), full attention for
8 (b, h) pairs, row-split W_proj partial product. Host sums the two
partials per batch and adds the (projected) biases.

All matmuls run in fp32r (4x fp32 throughput, ~1e-4 relative error).
Softmax is computed without max-subtraction (scores are O(1) for this
problem's scale) and without any partition-dim reduction: the exp'd
scores P^T live in [key, query] layout, so the denominator l[q] comes
out of the att@V matmul itself via a ones-column appended to V.

Phase 2 runs entirely in the (64,128) PE tile mode: QK matmuls for a
head PAIR run concurrently on row-tiles T0/T8 (contraction D=64), and
the att@V contraction (128 keys) is split into two 64-row halves that
accumulate into the same PSUM bank — no tile-mode switches (each switch
drains the PE), and the array's full 128 rows stay active so the HAM
clock gate keeps the PE at 2.4 GHz. Softmax normalization is deferred:
denominator rows collect into a [32, 512] tile, one batched reciprocal,
then per-(pair, j) broadcast matmuls with one-hot selector weights.
"""

import math
from contextlib import ExitStack

import numpy as np

import concourse.bass as bass
import concourse.mybir as mybir
from concourse import bacc
from concourse.bass_utils import run_bass_kernel_spmd
from concourse.masks import make_identity
from concourse.tile import TileContext

B, T, C = 4, 2048, 1024
H, D = 16, 64
HL = 8            # heads per core
NCORES = 8
P = 128
CK = C // P       # 8 contraction chunks for the QKV projection
TB = 512          # t-block (query-block) width
NTB = T // TB     # 4
NTT = T // P      # 16 row tiles
QKC = 2 * HL * D  # 1024 q+k channels per core
VC = HL * D       # 512 v channels per core
PC = VC           # 512 proj contraction channels per core
NPAIR = HL // 2   # 4 head pairs per core

f32 = mybir.dt.float32
f32r = mybir.dt.float32r


def _build_program():
    nc = bacc.Bacc("TRN2", target_bir_lowering=False, debug=False)
    # fp32r is bit-compatible with fp32 (HW rounds on read) — declaring the
    # inputs as fp32r lets DMA feed matmul tiles directly, no cast pass.
    x = nc.dram_tensor("x", (T, C), f32r, kind="ExternalInput").ap()
    wqkv = nc.dram_tensor("wqkv", (C, 3 * VC), f32r, kind="ExternalInput").ap()
    bqk = nc.dram_tensor("bqk", (P, CK), f32, kind="ExternalInput").ap()
    wproj = nc.dram_tensor("wproj", (PC, C), f32r, kind="ExternalInput").ap()
    # one-hot selector weights for the denominator broadcast matmuls:
    # sel[k, v, c] = 1 iff k == 2v + (c >= 64)
    sel = nc.dram_tensor("sel", (32, 2 * HL * P), f32r, kind="ExternalInput").ap()
    out = nc.dram_tensor("out", (T, C), f32, kind="ExternalOutput").ap()

    with TileContext(nc) as tc:
        with ExitStack() as ctx:
            # ---- persistent pools (whole kernel) ----
            const = ctx.enter_context(tc.tile_pool(name="const", bufs=1))
            persist = ctx.enter_context(tc.tile_pool(name="persist", bufs=1))

            identity = const.tile([P, P], f32)
            make_identity(nc, identity)
            identity_r = const.tile([P, P], f32r)
            nc.vector.tensor_copy(identity_r[:], identity[:])
            # causal mask bank: mw[p, i] = 1.0 iff i >= p + 512. The slice
            # mw[:, 512-dlt : 640] multiplied into exp'd scores zeroes
            # key > query entries for a chunk whose diagonal sits at dlt.
            mw = const.tile([P, 5 * P], f32)
            nc.gpsimd.memset(mw[:], 1.0)
            nc.gpsimd.affine_select(
                out=mw[:],
                in_=mw[:],
                compare_op=mybir.AluOpType.is_ge,
                fill=0.0,
                base=-512,
                pattern=[[1, 5 * P]],
                channel_multiplier=-1,
            )
            ones_f = const.tile([P, 1], f32)
            nc.gpsimd.memset(ones_f[:], 1.0)
            bqk_sb = const.tile([P, CK], f32)
            nc.sync.dma_start(bqk_sb[:], bqk)
            sel_sb = const.tile([32, 2 * HL, P], f32r)
            nc.sync.dma_start(sel_sb[:], sel.rearrange("k (v c) -> k v c", c=P))

            # V (no bias, fp32r) resident for phase 2: [t-part, tt, head, 64+1]
            vaug = persist.tile([P, NTT, HL, D + 1], f32r)
            nc.vector.tensor_copy(
                vaug[:, :, :, D : D + 1],
                ones_f[:, None, None, 0:1].to_broadcast((P, NTT, HL, 1)),
            )
            # softmax denominators, row 2*(4m+j) = head 2m, +1 = head 2m+1
            lrows = persist.tile([2 * NPAIR * NTB, TB], f32)

            with ExitStack() as c12:
                # Q^T/K^T resident across phases 1-2:
                # [128 rows = 2 heads x 64 d, jtile 0..3 = Q pairs,
                #  jtile 4..7 = K pairs, t]
                qkt_pool = c12.enter_context(tc.tile_pool(name="qkt", bufs=1))
                qkt = qkt_pool.tile([P, CK, T], f32r)

                # ---- phase 1: x^T, QKV^T ----
                with ExitStack() as c1:
                    wq_pool = c1.enter_context(tc.tile_pool(name="wq", bufs=1))
                    xin_pool = c1.enter_context(tc.tile_pool(name="xin", bufs=2))
                    xtr_pool = c1.enter_context(tc.tile_pool(name="xtr", bufs=2))
                    tp_psum = c1.enter_context(
                        tc.tile_pool(name="tp_psum", bufs=3, space="PSUM")
                    )
                    mm_psum = c1.enter_context(
                        tc.tile_pool(name="mm_psum", bufs=5, space="PSUM")
                    )

                    wqkv_r = wq_pool.tile([P, CK, 3 * VC], f32r)
                    for cc in range(CK):
                        nc.scalar.dma_start(
                            wqkv_r[:, cc, :], wqkv[cc * P : (cc + 1) * P, :]
                        )

                    def transpose_block(tb, xtr):
                        """Issue x loads for t-block tb; return closures that
                        each emit one PE transpose + ACT psum evacuation.
                        The closures are interleaved into the PREVIOUS
                        t-block's GEMM stream so the PE's MAC activity never
                        dips long enough for the HAM clock gate to throttle
                        the array back to 1.2 GHz."""
                        ops = []
                        for half in range(2):
                            t0 = tb * TB + half * 2 * P
                            xin = xin_pool.tile([P, 2, C], f32r)
                            nc.sync.dma_start(
                                xin[:],
                                x[t0 : t0 + 2 * P, :].rearrange(
                                    "(a p) c -> p a c", p=P
                                ),
                            )
                            for a in range(2):
                                for cc in range(CK):
                                    def op(xin=xin, a=a, cc=cc, half=half):
                                        tp = tp_psum.tile([P, P], f32r)
                                        nc.tensor.transpose(
                                            tp[:],
                                            xin[:, a, cc * P : (cc + 1) * P],
                                            identity_r[:],
                                        )
                                        # alternate evacuation engines so the
                                        # tp psum pool drains in parallel
                                        eng = nc.scalar.copy if cc % 2 else (
                                            nc.vector.tensor_copy
                                        )
                                        eng(
                                            xtr[
                                                :, cc,
                                                (half * 2 + a) * P
                                                : (half * 2 + a + 1) * P,
                                            ],
                                            tp[:],
                                        )
                                    ops.append(op)
                        return ops

                    def gemm_groups(tb, xtr):
                        groups = []
                        for j in range(QKC // P):
                            def gq(j=j, xtr=xtr):
                                ps = mm_psum.tile([P, TB], f32, tag="mm")
                                for cc in range(CK):
                                    nc.tensor.matmul(
                                        ps[:],
                                        wqkv_r[:, cc, j * P : (j + 1) * P],
                                        xtr[:, cc, :],
                                        start=(cc == 0),
                                        stop=(cc == CK - 1),
                                    )
                                nc.vector.tensor_scalar_add(
                                    qkt[:, j, tb * TB : (tb + 1) * TB],
                                    ps[:],
                                    bqk_sb[:, j : j + 1],
                                )
                            groups.append(gq)
                        for ts4 in range(TB // P):
                            def gv(ts4=ts4, tb=tb, xtr=xtr):
                                tt = tb * (TB // P) + ts4
                                ps = mm_psum.tile([P, VC], f32, tag="mm")
                                for cc in range(CK):
                                    nc.tensor.matmul(
                                        ps[:],
                                        xtr[:, cc, ts4 * P : (ts4 + 1) * P],
                                        wqkv_r[:, cc, QKC : QKC + VC],
                                        start=(cc == 0),
                                        stop=(cc == CK - 1),
                                    )
                                nc.vector.tensor_copy(
                                    vaug[:, tt, :, 0:D],
                                    ps[:].rearrange("p (h d) -> p h d", h=HL),
                                )
                            groups.append(gv)
                        return groups

                    xtr_cur = xtr_pool.tile([P, CK, TB], f32r, tag="xtr")
                    for op in transpose_block(0, xtr_cur):
                        op()
                    for tb in range(NTB):
                        if tb + 1 < NTB:
                            xtr_next = xtr_pool.tile([P, CK, TB], f32r, tag="xtr")
                            pend = transpose_block(tb + 1, xtr_next)
                        else:
                            xtr_next, pend = None, []
                        groups = gemm_groups(tb, xtr_cur)
                        per = -(-len(pend) // len(groups)) if pend else 0
                        for gi, g in enumerate(groups):
                            g()
                            for op in pend[gi * per : (gi + 1) * per]:
                                op()
                        xtr_cur = xtr_next

                # ---- phase 2: attention per head pair, (64,128) tile mode ----
                with ExitStack() as c2:
                    yt_pool = c2.enter_context(tc.tile_pool(name="yt", bufs=1))
                    # y^T (unnormalized): rows = head channel (2 heads per 128)
                    ytile = yt_pool.tile([P, NPAIR, T], f32r)
                    # prefetch W_proj during attention (ACT dma queue)
                    wp_pool = c2.enter_context(tc.tile_pool(name="wp", bufs=1))
                    wproj_r = wp_pool.tile([P, PC // P, C], f32r)
                    nc.scalar.dma_start(
                        wproj_r[:], wproj.rearrange("(a p) o -> p a o", p=P)
                    )

                    c2p = c2.enter_context(ExitStack())
                    pt_pool = c2p.enter_context(tc.tile_pool(name="pt", bufs=6))
                    st_pool = c2p.enter_context(tc.tile_pool(name="st", bufs=2))
                    scr_pool = c2p.enter_context(tc.tile_pool(name="scr", bufs=2))
                    ps_psum = c2p.enter_context(
                        tc.tile_pool(name="ps_psum", bufs=2, space="PSUM")
                    )
                    py_psum = c2p.enter_context(
                        tc.tile_pool(name="py_psum", bufs=1, space="PSUM")
                    )

                    scale = 1.0 / math.sqrt(D)
                    for m in range(NPAIR):
                        # head A = 2m on rows 0-63 / tile T0,
                        # head B = 2m+1 on rows 64-127 / tile T8
                        qT = qkt[:, m, :]
                        kT = qkt[:, 4 + m, :]
                        for j in range(NTB):
                            nch = 4 * j + 4  # causal: key chunks 0..4j+3
                            pyA0 = py_psum.tile([D + 1, TB], f32, tag="pyA0")
                            pyA1 = py_psum.tile([D + 1, TB], f32, tag="pyA1")
                            pyB0 = py_psum.tile([D + 1, TB], f32, tag="pyB0")
                            pyB1 = py_psum.tile([D + 1, TB], f32, tag="pyB1")
                            pts = []

                            def dstart(c, j=j):
                                return max(0, (c - 4 * j) * P)

                            def av_pair(c, last):
                                # att@V, contraction split across row-tiles
                                # T0 (keys 0-63) / T8 (keys 64-127) into
                                # separate PSUM tiles (cross-tile PSUM
                                # accumulation crashes); A-T0 || B-T8 then
                                # A-T8 || B-T0 keeps both tiles streaming.
                                d0 = dstart(c)
                                pt = pts[c]
                                first = c == 0
                                nc.tensor.matmul(
                                    pyA0[:, d0:],
                                    vaug[0:64, c, 2 * m, :],
                                    pt[0:64, 0, d0:],
                                    start=first, stop=last,
                                )
                                nc.tensor.matmul(
                                    pyB1[:, d0:],
                                    vaug[64:128, c, 2 * m + 1, :],
                                    pt[64:128, 1, d0:],
                                    start=first, stop=last,
                                )
                                nc.tensor.matmul(
                                    pyA1[:, d0:],
                                    vaug[64:128, c, 2 * m, :],
                                    pt[64:128, 0, d0:],
                                    start=first, stop=last,
                                )
                                nc.tensor.matmul(
                                    pyB0[:, d0:],
                                    vaug[0:64, c, 2 * m + 1, :],
                                    pt[0:64, 1, d0:],
                                    start=first, stop=last,
                                )

                            for c in range(nch):
                                d0 = dstart(c)
                                ps = ps_psum.tile([P, 2, TB], f32)
                                # QK for both heads: concurrent row-tiles
                                nc.tensor.matmul(
                                    ps[:, 0, d0:],
                                    kT[0:64, c * P : (c + 1) * P],
                                    qT[0:64, j * TB + d0 : (j + 1) * TB],
                                    start=True, stop=True,
                                )
                                nc.tensor.matmul(
                                    ps[:, 1, d0:],
                                    kT[64:128, c * P : (c + 1) * P],
                                    qT[64:128, j * TB + d0 : (j + 1) * TB],
                                    start=True, stop=True,
                                )
                                pt = pt_pool.tile([P, 2, TB], f32r)
                                nc.scalar.activation(
                                    pt[:, :, d0:], ps[:, :, d0:],
                                    mybir.ActivationFunctionType.Exp, scale=scale,
                                )
                                if (c - 4 * j) * P >= 0:
                                    # zero key > query entries on the diagonal
                                    # (gpsimd: keeps the DVE off this chain)
                                    for s_ in range(2):
                                        nc.gpsimd.affine_select(
                                            out=pt[:, s_, d0 : d0 + P],
                                            in_=pt[:, s_, d0 : d0 + P],
                                            compare_op=mybir.AluOpType.is_ge,
                                            fill=0.0,
                                            base=0,
                                            pattern=[[1, P]],
                                            channel_multiplier=-1,
                                        )
                                pts.append(pt)
                                # trail AV by 4 chunks: queues enough QK/exp
                                # ahead of the py-gated first AV that the
                                # in-order PE queue doesn't head-block (and
                                # starve ACT) while the previous block's
                                # evacuation frees the py tiles.
                                if c >= 4:
                                    av_pair(c - 4, last=False)
                            for c in range(max(0, nch - 4), nch):
                                av_pair(c, last=(c == nch - 1))

                            # stash denominator rows: engine-copy to a
                            # partition-0 staging tile (engines can't write at
                            # arbitrary partition offsets), then DMA-scatter
                            # into lrows partitions r, r+1
                            # evacuate each T0-half once (ACT takes head A
                            # incl. its denominator row, DVE head B), then
                            # fused adds with the T8-halves write ytile and
                            # the denominator staging in single passes.
                            r = 2 * (4 * m + j)
                            scrA = scr_pool.tile([D + 1, TB], f32, tag="sA")
                            scrB = scr_pool.tile([D + 1, TB], f32, tag="sB")
                            nc.scalar.copy(scrA[:], pyA0[:])
                            nc.vector.tensor_copy(scrB[:], pyB0[:])
                            st = st_pool.tile([1, 2, TB], f32)
                            nc.vector.tensor_add(
                                st[0:1, 0, :], scrA[D : D + 1, :],
                                pyA1[D : D + 1, :],
                            )
                            nc.vector.tensor_add(
                                st[0:1, 1, :], scrB[D : D + 1, :],
                                pyB1[D : D + 1, :],
                            )
                            nc.sync.dma_start(
                                lrows[r : r + 2, :],
                                st[0:1, :, :],
                            )
                            ysA = ytile[0:64, m, j * TB : (j + 1) * TB]
                            nc.vector.tensor_add(ysA, scrA[0:D, :], pyA1[0:D, :])
                            ysB = ytile[64:128, m, j * TB : (j + 1) * TB]
                            nc.vector.tensor_add(ysB, scrB[0:D, :], pyB1[0:D, :])

                    c2p.close()

                    # ---- normalize: one batched reciprocal, then per-(m,j)
                    # broadcast matmuls with one-hot selector weights ----
                    with ExitStack() as cn:
                        sm_pool = cn.enter_context(tc.tile_pool(name="sm", bufs=1))
                        lb_psum = cn.enter_context(
                            tc.tile_pool(name="lb_psum", bufs=4, space="PSUM")
                        )
                        linv_r = sm_pool.tile([2 * NPAIR * NTB, TB], f32r)
                        # f32r output is bit-identical to f32 — no precision loss
                        with nc.allow_low_precision(reason="f32r == f32 bits"):
                            nc.vector.reciprocal(linv_r[:], lrows[:])
                        for m in range(NPAIR):
                            for j in range(NTB):
                                v = 4 * m + j
                                lb = lb_psum.tile([P, TB], f32)
                                # lb rows 0-63 = 1/l_A, 64-127 = 1/l_B
                                nc.tensor.matmul(
                                    lb[:], sel_sb[:, v, :], linv_r[:],
                                    start=True, stop=True,
                                )
                                nc.vector.tensor_mul(
                                    ytile[:, m, j * TB : (j + 1) * TB],
                                    ytile[:, m, j * TB : (j + 1) * TB],
                                    lb[:],
                                )

                    # ---- phase 3: projection (row-split partial product) ----
                    with ExitStack() as c3:
                        ot_pool = c3.enter_context(tc.tile_pool(name="ot", bufs=3))
                        po_psum = c3.enter_context(
                            tc.tile_pool(name="po_psum", bufs=4, space="PSUM")
                        )
                        for t2 in range(NTT // 2):
                            ot = ot_pool.tile([P, 2, C], f32)
                            for a2 in range(2):
                                tt = 2 * t2 + a2
                                for nh in range(C // TB):
                                    po = po_psum.tile([P, TB], f32)
                                    for a in range(PC // P):
                                        nc.tensor.matmul(
                                            po[:],
                                            ytile[:, a, tt * P : (tt + 1) * P],
                                            wproj_r[
                                                :, a, nh * TB : (nh + 1) * TB
                                            ],
                                            start=(a == 0),
                                            stop=(a == PC // P - 1),
                                        )
                                    nc.vector.tensor_copy(
                                        ot[:, a2, nh * TB : (nh + 1) * TB], po[:]
                                    )
                            (nc.sync if t2 % 2 == 0 else nc.scalar).dma_start(
                                out[2 * t2 * P : (2 * t2 + 2) * P, :].rearrange(
                                    "(a p) c -> p a c", p=P
                                ),
                                ot[:],
                            )

    nc.compile()
    return nc


_NC_CACHE = None


def _get_program():
    global _NC_CACHE
    if _NC_CACHE is None:
        _NC_CACHE = _build_program()
    return _NC_CACHE


def _sel_matrix():
    # sel[k, v, c] = 1 iff k == 2v + (c >= 64); broadcast-matmul weights
    # that map denominator rows [32, TB] to a per-(pair, j) [128, TB] tile.
    s = np.zeros((32, 2 * HL, P), dtype=np.float32)
    for v in range(2 * HL):
        s[2 * v, v, 0:64] = 1.0
        s[2 * v + 1, v, 64:128] = 1.0
    return np.ascontiguousarray(s.reshape(32, 2 * HL * P))


def _shard_inputs(x, W_attn, b_attn, bQ, bK, bV, W_proj):
    sel = _sel_matrix()
    in_maps = []
    for c in range(NCORES):
        b = c // 2
        half = c % 2
        s = half * VC
        wq = W_attn[:, s : s + VC]
        wk = W_attn[:, C + s : C + s + VC]
        wv = W_attn[:, 2 * C + s : 2 * C + s + VC]
        wqkv = np.ascontiguousarray(np.concatenate([wq, wk, wv], axis=1))
        bq = b_attn[s : s + VC] + bQ[half * HL : half * HL + HL].reshape(-1)
        bk = b_attn[C + s : C + s + VC] + bK[half * HL : half * HL + HL].reshape(-1)
        bqk = np.ascontiguousarray(
            np.concatenate([bq, bk]).reshape(CK, P).T.astype(np.float32)
        )
        wproj = np.ascontiguousarray(W_proj[s : s + VC, :])
        in_maps.append(
            {
                "x": np.ascontiguousarray(x[b]),
                "wqkv": wqkv,
                "bqk": bqk,
                "wproj": wproj,
                "sel": sel,
            }
        )
    return in_maps


def kernel(x, W_attn, b_attn, W_proj, b_proj, bQ, bK, bV, _trace=False, _res_out=None):
    x = np.asarray(x, dtype=np.float32)
    W_attn = np.asarray(W_attn, dtype=np.float32)
    b_attn = np.asarray(b_attn, dtype=np.float32)
    W_proj = np.asarray(W_proj, dtype=np.float32)
    b_proj = np.asarray(b_proj, dtype=np.float32)
    bQ = np.asarray(bQ, dtype=np.float32)
    bK = np.asarray(bK, dtype=np.float32)
    bV = np.asarray(bV, dtype=np.float32)

    nc = _get_program()
    in_maps = _shard_inputs(x, W_attn, b_attn, bQ, bK, bV, W_proj)
    res = run_bass_kernel_spmd(
        nc, in_maps, core_ids=list(range(NCORES)), trace=_trace
    )
    if _res_out is not None:
        _res_out.append(res)

    # v-bias passes through softmax untouched (rows of att sum to 1), so it
    # projects to a constant vector; fold it with b_proj on the host.
    bv = b_attn[2 * C : 3 * C] + bV.reshape(-1)
    extra = bv @ W_proj + b_proj
    out = np.empty((B, T, C), dtype=np.float32)
    for b in range(B):
        out[b] = res.results[2 * b]["out"] + res.results[2 * b + 1]["out"] + extra
    return out
